# revision 16
# baseline (speedup 1.0000x reference)
"""Trainium2 Bass kernel for nn_Attention_76450417868987.

Module: three Bahdanau-style additive attentions + gated fusion.
Sharding: pure data-parallel, batch 512 -> 64 per core across 8 cores.

Layout strategy (per core, everything f32):
  - Big tensors stream in natural layout: flattened [(b n), d] tiles of
    [128, 512] (d contiguous -> efficient DMA).
  - h_proj broadcast-add done on TensorE: X = indicator.T @ hp + I.T @ P
    accumulated in PSUM (indicator selects the <=3 batch rows a tile spans).
  - tanh on ScalarE straight out of PSUM.
  - score = sum_d tanh(X) * aw via ONE fused DVE tensor_tensor_reduce.
  - scores bounce through DRAM to re-chunk from (b n)-flat to [64, n]
    batch-on-partition layout; softmax = reduce_max(negate) + Exp(accum_out)
    + reciprocal + tensor_scalar_mul.
  - attention-weighted feature sum: block-diagonal weight tiles [128, 64]
    (one column per batch item) x feature tiles [128, 512] accumulated over
    all tiles into a single PSUM bank -> res [64, 512] with no per-b copies.
  - gate: PE transposes of cont/senti (h already transposed), 20 matmuls +
    bias matmul into PSUM, tanh, fused alpha-dot via tensor_tensor_reduce,
    sigmoid, final blend via tensor_scalar ops.
"""

import os
import sys

if "/opt/trn_rl_repo" not in sys.path:
    sys.path.insert(0, "/opt/trn_rl_repo")

import numpy as np

B = 512
NA, NCP, NSW = 196, 50, 50
D = 512
M = 8
BL = B // M  # 64
NT_A = BL * NA // 128  # 98
NT_C = BL * NCP // 128  # 25
P = 128

_CACHE = {}


def _segs(per_n):
    """Per flattened tile t of [BL*per_n, D]: list of (row_off, run_len, b)."""
    segs = []
    for t in range(BL * per_n // 128):
        lst = []
        r = t * 128
        while r < t * 128 + 128:
            b = r // per_n
            e = min((b + 1) * per_n, t * 128 + 128)
            lst.append((r - t * 128, e - r, b))
            r = e
        segs.append(lst)
    return segs


SEGS_A = _segs(NA)
SEGS_C = _segs(NCP)
KMAX_A = max(len(s) for s in SEGS_A)  # 2
KMAX_C = max(len(s) for s in SEGS_C)  # 3


def _ind_const(per_n, kmax):
    segs = _segs(per_n)
    nt = len(segs)
    a = np.zeros((kmax, nt * 128), np.float32)
    for t, lst in enumerate(segs):
        b0 = lst[0][2]
        for (off, ln, b) in lst:
            a[b - b0, t * 128 + off : t * 128 + off + ln] = 1.0
    return a


def _build(nc):
    import concourse.bass as bass  # noqa: F401
    from concourse import mybir
    from concourse.tile import TileContext

    f32 = mybir.dt.float32
    AF = mybir.ActivationFunctionType
    OP = mybir.AluOpType
    AX = mybir.AxisListType

    def dp(name, shape):
        return nc.declare_dram_parameter(name, shape, f32, isOutput=False)

    h_d = dp("h", [BL, D])
    att_f = dp("att_feats", [BL * NA, D])
    p_att = dp("p_att_feats", [BL * NA, D])
    cpt_f = dp("cpt_feats", [BL * NCP, D])
    p_cpt = dp("p_cpt_feats", [BL * NCP, D])
    senti_d = dp("senti_feats", [BL, D])
    sw_f = dp("senti_word_feats", [BL * NSW, D])
    p_sw = dp("p_senti_word_feats", [BL * NSW, D])

    w_h2att = dp("c_h2att_w", [D, D])
    b_h2att = dp("c_h2att_b", [1, D])
    w_h2cpt = dp("c_h2cpt_w", [D, D])
    b_h2cpt = dp("c_h2cpt_b", [1, D])
    aw_att_d = dp("c_attA_w", [1, D])
    aw_cpt_d = dp("c_cptA_w", [1, D])
    w_h2sw = dp("s_h2word_w", [D, D])
    b_h2sw = dp("s_h2word_b", [1, D])
    aw_sw_d = dp("s_wordA_w", [1, D])
    w_th = dp("t_h2att_w", [D, D])
    b_th = dp("t_h2att_b", [1, D])
    w_tc = dp("t_cont_w", [2 * D, D])
    b_tc = dp("t_cont_b", [1, D])
    w_ts = dp("t_senti_w", [2 * D, D])
    b_ts = dp("t_senti_b", [1, D])
    w_ta_d = dp("t_alpha_w", [1, D])
    b_ta_d = dp("t_alpha_b", [1, 1])

    ident_d = dp("ident", [P, P])
    ones4_d = dp("ones4", [4, P])
    ind_att_d = dp("ind_att", [KMAX_A, NT_A * 128])
    ind_50_d = dp("ind_50", [KMAX_C, NT_C * 128])

    out_d = nc.declare_dram_parameter("out", [BL, 2 * D], f32, isOutput=True)

    # DRAM scratch for score re-chunking (flat (b n) order).
    sflat = {
        "a": nc.dram_tensor("sflat_a", [NT_A, 128, 1], f32),
        "c": nc.dram_tensor("sflat_c", [NT_C, 128, 1], f32),
        "s": nc.dram_tensor("sflat_s", [NT_C, 128, 1], f32),
    }
    wflat = {
        "a": nc.dram_tensor("wflat_a", [NT_A, 128, 1], f32),
        "c": nc.dram_tensor("wflat_c", [NT_C, 128, 1], f32),
        "s": nc.dram_tensor("wflat_s", [NT_C, 128, 1], f32),
    }
    # hp bounced to DRAM so per-tile row-slices can be DMA'd to partition 0
    # (matmul operands require 32-aligned matching base partitions).
    hp_dram = {
        "a": nc.dram_tensor("hp_dram_a", [BL, D], f32),
        "c": nc.dram_tensor("hp_dram_c", [BL, D], f32),
        "s": nc.dram_tensor("hp_dram_s", [BL, D], f32),
    }

    with TileContext(nc) as tc:
        with (
            tc.tile_pool(name="const", bufs=1) as constp,
            tc.tile_pool(name="pio", bufs=6) as piop,
            tc.tile_pool(name="sio", bufs=4) as siop,
            tc.tile_pool(name="fio", bufs=10) as fiop,
            tc.tile_pool(name="work", bufs=3) as workp,
            tc.tile_pool(name="small", bufs=2) as smallp,
            tc.tile_pool(name="big", bufs=1) as bigp,
            tc.tile_pool(name="psx", bufs=2, space="PSUM") as psxp,
            tc.tile_pool(name="psres", bufs=2, space="PSUM") as psresp,
            tc.tile_pool(name="psaux", bufs=3, space="PSUM") as psauxp,
        ):
            # ---------------- setup ----------------
            ident = constp.tile([P, P], f32, tag="ident")
            nc.sync.dma_start(ident[:], ident_d[:])
            ones4 = constp.tile([4, P], f32, tag="ones4")
            nc.sync.dma_start(ones4[:], ones4_d[:])
            h_sb = constp.tile([BL, D], f32, tag="h_sb")
            nc.sync.dma_start(h_sb[:], h_d[:])

            # hT[:, c, :] = h[:, 128c:128(c+1)].T  (PE transpose)
            hT = constp.tile([P, 4, BL], f32, tag="hT")
            for c in range(4):
                tp = psauxp.tile([P, BL], f32, tag="aux")
                nc.tensor.transpose(tp[:], h_sb[:, c * P : (c + 1) * P], ident[:BL, :BL])
                nc.scalar.copy(hT[:, c, :], tp[:])

            bf16 = mybir.dt.bfloat16

            def bcast_row(dram_row, tag, dtype=f32):
                """-> sbuf [128, D] with every partition = the dram row."""
                row = smallp.tile([1, D], f32, tag="brow")
                nc.sync.dma_start(row[:], dram_row[:1, :])
                ps = psauxp.tile([P, D], f32, tag="aux")
                nc.tensor.matmul(ps[:], ones4[:1, :], row[:], start=True, stop=True)
                sb = constp.tile([P, D], dtype, tag=tag)
                nc.scalar.copy(sb[:], ps[:])
                return sb

            awb = {
                "a": bcast_row(aw_att_d, "awb_a", bf16),
                "c": bcast_row(aw_cpt_d, "awb_c", bf16),
                "s": bcast_row(aw_sw_d, "awb_s", bf16),
            }
            alphab = bcast_row(w_ta_d, "alphab")

            ab_sb = smallp.tile([1, 1], f32, tag="ab_sb")
            nc.sync.dma_start(ab_sb[:], b_ta_d[:])
            ps = psauxp.tile([BL, 1], f32, tag="aux")
            nc.tensor.matmul(ps[:], ones4[:1, :BL], ab_sb[:], start=True, stop=True)
            ab_col = constp.tile([BL, 1], f32, tag="ab_col")
            nc.scalar.copy(ab_col[:], ps[:])

            def proj(wd, bd, tag):
                """hp = h @ W + b -> sbuf [64, 512]."""
                hp_ps = psauxp.tile([BL, D], f32, tag="aux")
                for c in range(4):
                    wt = workp.tile([P, D], f32, tag="wproj")
                    nc.sync.dma_start(wt[:], wd[c * P : (c + 1) * P, :])
                    nc.tensor.matmul(
                        hp_ps[:], hT[:, c, :], wt[:], start=(c == 0), stop=False
                    )
                brow = smallp.tile([1, D], f32, tag="brow")
                nc.sync.dma_start(brow[:], bd[:1, :])
                nc.tensor.matmul(hp_ps[:], ones4[:1, :BL], brow[:], start=False, stop=True)
                sb = constp.tile([BL, D], f32, tag=tag)
                nc.scalar.copy(sb[:], hp_ps[:])
                return sb

            hp = {
                "a": proj(w_h2att, b_h2att, "hp_a"),
                "c": proj(w_h2cpt, b_h2cpt, "hp_c"),
                "s": proj(w_h2sw, b_h2sw, "hp_s"),
            }
            for key in ("a", "c", "s"):
                nc.scalar.dma_start(hp_dram[key][:], hp[key][:])

            # ---------------- score phase ----------------
            def score_branch(key, p_dram, per_n, nt, ind_dram, segs):
                for t in range(nt):
                    pt = piop.tile([P, D], f32, tag="p_in")
                    nc.sync.dma_start(pt[:], p_dram[t * 128 : (t + 1) * 128, :])
                    b0 = segs[t][0][2]
                    k = len(segs[t])
                    hpe = siop.tile([KMAX_C, D], f32, tag="hp_in")
                    nc.scalar.dma_start(hpe[:k, :], hp_dram[key][b0 : b0 + k, :])
                    ind_t = siop.tile([KMAX_C, 128], f32, tag="ind_in")
                    nc.scalar.dma_start(
                        ind_t[:k, :], ind_dram[:k, t * 128 : (t + 1) * 128]
                    )
                    xps = psxp.tile([P, D], f32, tag="xps")
                    nc.tensor.matmul(xps[:], ident[:], pt[:], start=True, stop=False)
                    nc.tensor.matmul(
                        xps[:],
                        ind_t[:k, :],
                        hpe[:k, :],
                        start=False,
                        stop=True,
                    )
                    tt = workp.tile([P, D], bf16, tag="tanh")
                    nc.scalar.activation(tt[:], xps[:], AF.Tanh)
                    prod = workp.tile([P, D], bf16, tag="prod")
                    nc.vector.tensor_mul(prod[:], tt[:], awb[key][:])
                    scol = workp.tile([P, 1], f32, tag="scol")
                    nc.vector.tensor_reduce(scol[:], prod[:], axis=AX.X, op=OP.add)
                    nc.scalar.dma_start(sflat[key][t], scol[:])

            def softmax_w(key, n, nt):
                """scores (dram flat) -> softmax weights [64, n] sbuf."""
                bview = (
                    sflat[key][:, :, 0]
                    .rearrange("t p -> (t p)")
                    .rearrange("(b n) -> b n", b=BL)
                )
                sc = smallp.tile([BL, n], f32, tag=f"sc_{key}")
                nc.sync.dma_start(sc[:], bview)
                mneg = smallp.tile([BL, 1], f32, tag=f"mneg_{key}")
                nc.vector.tensor_reduce(
                    mneg[:], sc[:], axis=AX.X, op=OP.max, negate=True
                )
                e = smallp.tile([BL, n], f32, tag=f"e_{key}")
                ssum = smallp.tile([BL, 1], f32, tag=f"ssum_{key}")
                nc.scalar.activation(
                    e[:], sc[:], AF.Exp, bias=mneg[:], scale=1.0, accum_out=ssum[:]
                )
                rec = smallp.tile([BL, 1], f32, tag=f"rec_{key}")
                nc.vector.reciprocal(rec[:], ssum[:])
                w = smallp.tile([BL, n], f32, tag=f"w_{key}")
                nc.vector.tensor_scalar_mul(w[:], e[:], rec[:])
                return w

            def lhst_build(key, w_sb, nt, segs):
                """softmax w [64, n] -> block-diag lhsT stack [128, nt*64]."""
                bview = (
                    wflat[key][:, :, 0]
                    .rearrange("t p -> (t p)")
                    .rearrange("(b n) -> b n", b=BL)
                )
                nc.scalar.dma_start(bview, w_sb[:])
                lt = bigp.tile([P, nt * BL], f32, tag=f"lt_{key}")
                nc.vector.memset(lt[:], 0.0)
                for t in range(nt):
                    for (off, ln, b) in segs[t]:
                        nc.scalar.dma_start(
                            lt[off : off + ln, t * BL + b : t * BL + b + 1],
                            wflat[key][t, off : off + ln, :],
                        )
                return lt

            score_branch("a", p_att, NA, NT_A, ind_att_d, SEGS_A)
            w_a = softmax_w("a", NA, NT_A)
            lt_a = lhst_build("a", w_a, NT_A, SEGS_A)

            score_branch("c", p_cpt, NCP, NT_C, ind_50_d, SEGS_C)
            w_c = softmax_w("c", NCP, NT_C)
            lt_c = lhst_build("c", w_c, NT_C, SEGS_C)

            score_branch("s", p_sw, NSW, NT_C, ind_50_d, SEGS_C)
            w_s = softmax_w("s", NSW, NT_C)
            lt_s = lhst_build("s", w_s, NT_C, SEGS_C)

            # ---------------- einsum phase ----------------
            def einsum_branch(f_dram, lt, nt):
                res = psresp.tile([BL, D], f32, tag="res")
                for t in range(nt):
                    ft = fiop.tile([P, D], f32, tag="f_in")
                    nc.sync.dma_start(ft[:], f_dram[t * 128 : (t + 1) * 128, :])
                    nc.tensor.matmul(
                        res[:],
                        lt[:, t * BL : (t + 1) * BL],
                        ft[:],
                        start=(t == 0),
                        stop=(t == nt - 1),
                    )
                return res

            cont = constp.tile([BL, 2 * D], f32, tag="cont")
            res_a = einsum_branch(att_f, lt_a, NT_A)
            nc.scalar.copy(cont[:, :D], res_a[:])
            res_c = einsum_branch(cpt_f, lt_c, NT_C)
            nc.scalar.copy(cont[:, D:], res_c[:])

            sent = constp.tile([BL, 2 * D], f32, tag="sent")
            nc.sync.dma_start(sent[:, :D], senti_d[:])
            res_s = einsum_branch(sw_f, lt_s, NT_C)
            nc.scalar.copy(sent[:, D:], res_s[:])

            # ---------------- gate ----------------
            g_ps = psauxp.tile([BL, D], f32, tag="aux")
            first = True
            for (src_sb, wd, nch) in ((cont, w_tc, 8), (sent, w_ts, 8), (None, w_th, 4)):
                for c in range(nch):
                    if src_sb is None:
                        lhsT_c = hT[:, c, :]
                    else:
                        tp = psauxp.tile([P, BL], f32, tag="aux")
                        nc.tensor.transpose(
                            tp[:], src_sb[:, c * P : (c + 1) * P], ident[:BL, :BL]
                        )
                        ct = workp.tile([P, BL], f32, tag="gT")
                        nc.scalar.copy(ct[:], tp[:])
                        lhsT_c = ct[:]
                    wt = workp.tile([P, D], f32, tag="wproj")
                    nc.sync.dma_start(wt[:], wd[c * P : (c + 1) * P, :])
                    nc.tensor.matmul(g_ps[:], lhsT_c, wt[:], start=first, stop=False)
                    first = False
            b3 = smallp.tile([3, D], f32, tag="b3")
            nc.sync.dma_start(b3[0:1, :], b_tc[:1, :])
            nc.sync.dma_start(b3[1:2, :], b_ts[:1, :])
            nc.sync.dma_start(b3[2:3, :], b_th[:1, :])
            nc.tensor.matmul(g_ps[:], ones4[:3, :BL], b3[:], start=False, stop=True)

            g_sb = workp.tile([BL, D], f32, tag="g_sb")
            nc.scalar.activation(g_sb[:], g_ps[:], AF.Tanh)
            gprod = workp.tile([BL, D], f32, tag="gprod")
            nc.vector.tensor_mul(gprod[:], g_sb[:], alphab[:BL, :])
            gacc = smallp.tile([BL, 1], f32, tag="gacc")
            nc.vector.tensor_reduce(gacc[:], gprod[:], axis=AX.X, op=OP.add)
            gate = smallp.tile([BL, 1], f32, tag="gate")
            nc.scalar.activation(gate[:], gacc[:], AF.Sigmoid, bias=ab_col[:])

            diff = constp.tile([BL, 2 * D], f32, tag="diff")
            nc.vector.tensor_sub(diff[:], cont[:], sent[:])
            prd = constp.tile([BL, 2 * D], f32, tag="prd")
            nc.vector.tensor_scalar_mul(prd[:], diff[:], gate[:, 0:1])
            fin = constp.tile([BL, 2 * D], f32, tag="fin")
            nc.vector.tensor_add(fin[:], sent[:], prd[:])
            nc.sync.dma_start(out_d[:], fin[:])

    return nc


def _fixup_multiwait(nc):
    """This walrus build allows only ONE sync wait per instruction (except
    InstEventSemaphore). Split extra waits onto same-engine NOPs in front."""
    from concourse import mybir

    nfix = 0
    for fn in nc.m.functions:
        for blk in fn.blocks:
            new = []
            for inst in blk.instructions:
                si = inst.sync_info
                waits = list(si.on_wait) if si is not None else []
                if len(waits) > 1 and type(inst).__name__ != "InstEventSemaphore":
                    for w in waits[:-1]:
                        nop = mybir.InstNoOp(
                            name=nc.get_next_instruction_name(), ins=[], outs=[]
                        )
                        nop.engine = inst.engine
                        nop.sync_info = mybir.SyncInfo(on_wait=[w], on_update=[])
                        nc.register_instruction(nop)
                        new.append(nop)
                        nfix += 1
                    si.on_wait = waits[-1:]
                new.append(inst)
            blk.instructions[:] = new
    return nfix


def _get_nc():
    if "nc" not in _CACHE:
        import concourse.bass as bass

        nc = bass.Bass()
        _build(nc)
        nc.finalize()
        _fixup_multiwait(nc)
        _CACHE["nc"] = nc
    return _CACHE["nc"]


def _make_in_maps(inputs):
    f = lambda x: np.ascontiguousarray(np.asarray(x), dtype=np.float32)
    consts = {
        "ident": np.eye(P, dtype=np.float32),
        "ones4": np.ones((4, P), np.float32),
        "ind_att": _ind_const(NA, KMAX_A),
        "ind_50": _ind_const(NCP, KMAX_C),
    }
    weights = {
        "c_h2att_w": f(inputs["c_h2att_w"]),
        "c_h2att_b": f(inputs["c_h2att_b"]).reshape(1, D),
        "c_h2cpt_w": f(inputs["c_h2cpt_w"]),
        "c_h2cpt_b": f(inputs["c_h2cpt_b"]).reshape(1, D),
        "c_attA_w": f(inputs["c_attA_w"]).reshape(1, D),
        "c_cptA_w": f(inputs["c_cptA_w"]).reshape(1, D),
        "s_h2word_w": f(inputs["s_h2word_w"]),
        "s_h2word_b": f(inputs["s_h2word_b"]).reshape(1, D),
        "s_wordA_w": f(inputs["s_wordA_w"]).reshape(1, D),
        "t_h2att_w": f(inputs["t_h2att_w"]),
        "t_h2att_b": f(inputs["t_h2att_b"]).reshape(1, D),
        "t_cont_w": f(inputs["t_cont_w"]),
        "t_cont_b": f(inputs["t_cont_b"]).reshape(1, D),
        "t_senti_w": f(inputs["t_senti_w"]),
        "t_senti_b": f(inputs["t_senti_b"]).reshape(1, D),
        "t_alpha_w": f(inputs["t_alpha_w"]).reshape(1, D),
        "t_alpha_b": f(inputs["t_alpha_b"]).reshape(1, 1),
    }
    in_maps = []
    for i in range(M):
        sl = slice(i * BL, (i + 1) * BL)
        m = {
            "h": f(inputs["h"][sl]),
            "att_feats": f(inputs["att_feats"][sl]).reshape(BL * NA, D),
            "p_att_feats": f(inputs["p_att_feats"][sl]).reshape(BL * NA, D),
            "cpt_feats": f(inputs["cpt_feats"][sl]).reshape(BL * NCP, D),
            "p_cpt_feats": f(inputs["p_cpt_feats"][sl]).reshape(BL * NCP, D),
            "senti_feats": f(inputs["senti_feats"][sl]),
            "senti_word_feats": f(inputs["senti_word_feats"][sl]).reshape(BL * NSW, D),
            "p_senti_word_feats": f(inputs["p_senti_word_feats"][sl]).reshape(
                BL * NSW, D
            ),
        }
        m.update(weights)
        m.update(consts)
        in_maps.append(m)
    return in_maps


def _run(inputs, trace=False):
    from concourse.bass_utils import run_bass_kernel_spmd

    nc = _get_nc()
    in_maps = _make_in_maps(inputs)
    r = run_bass_kernel_spmd(nc, in_maps, core_ids=list(range(M)), trace=trace)
    out = np.concatenate([r.results[i]["out"] for i in range(M)], axis=0)
    return out, r


def kernel(**inputs):
    out, _ = _run(inputs, trace=False)
    return out


def _timed_runner(nc, in_maps, iters):
    """Build a runner for nc that executes with device-resident inputs and
    pre-staged donated output buffers; returns (per-call wall times ns, out)."""
    import time

    import jax
    from jax.sharding import Mesh, NamedSharding, PartitionSpec

    try:
        from jax.experimental.shard_map import shard_map
    except ImportError:
        from jax.shard_map import shard_map

    from concourse import bass2jax, mybir
    from concourse.bass2jax import _bass_exec_p

    bass2jax.install_neuronx_cc_hook()
    partition_name = nc.partition_id_tensor.name if nc.partition_id_tensor else None

    in_names, out_names, out_avals, zero_outs = [], [], [], []
    for alloc in nc.m.functions[0].allocations:
        if not isinstance(alloc, mybir.MemoryLocationSet):
            continue
        name = alloc.memorylocations[0].name
        if alloc.kind == "ExternalInput":
            if name != partition_name:
                in_names.append(name)
        elif alloc.kind == "ExternalOutput":
            out_names.append(name)
            out_avals.append(
                jax.core.ShapedArray(
                    tuple(alloc.tensor_shape), mybir.dt.np(alloc.dtype)
                )
            )
            zero_outs.append(
                np.zeros(tuple(alloc.tensor_shape), mybir.dt.np(alloc.dtype))
            )
    n_params = len(in_names)
    n_outs = len(out_names)
    all_in = list(in_names) + list(out_names)
    if partition_name:
        all_in.append(partition_name)

    def _body(*args):
        operands = list(args)
        if partition_name:
            operands.append(bass2jax.partition_id_tensor())
        return tuple(
            _bass_exec_p.bind(
                *operands,
                out_avals=tuple(out_avals),
                in_names=tuple(all_in),
                out_names=tuple(out_names),
                lowering_input_output_aliases=(),
                sim_require_finite=False,
                sim_require_nnan=False,
                nc=nc,
            )
        )

    devices = jax.devices()[:M]
    mesh = Mesh(np.asarray(devices), ("core",))
    donate = tuple(range(n_params, n_params + n_outs))
    sharded = jax.jit(
        shard_map(
            _body,
            mesh=mesh,
            in_specs=(PartitionSpec("core"),) * (n_params + n_outs),
            out_specs=(PartitionSpec("core"),) * n_outs,
            check_rep=False,
        ),
        donate_argnums=donate,
        keep_unused=True,
    )
    sh = NamedSharding(mesh, PartitionSpec("core"))
    per_core = [[np.asarray(m[name]) for name in in_names] for m in in_maps]
    args = [
        jax.device_put(
            np.concatenate([per_core[c][i] for c in range(M)], axis=0), sh
        )
        for i in range(n_params)
    ]
    # one donated zero-output set per call, staged up front
    zsets = []
    for _ in range(iters + 1):
        zsets.append(
            [jax.device_put(np.concatenate([z] * M, axis=0), sh) for z in zero_outs]
        )
    out = sharded(*args, *zsets[-1])
    jax.block_until_ready(out)
    times = []
    for i in range(iters):
        t0 = time.perf_counter()
        out = sharded(*args, *zsets[i])
        jax.block_until_ready(out)
        t1 = time.perf_counter()
        times.append((t1 - t0) * 1e9)
    return times, np.asarray(out[0])


def _tiny_nc():
    """Minimal NEFF for measuring the per-execute dispatch round-trip."""
    import concourse.bass as bass
    from concourse import mybir
    from concourse.tile import TileContext

    f32 = mybir.dt.float32
    nc = bass.Bass()
    a_d = nc.declare_dram_parameter("a", [P, D], f32, isOutput=False)
    o_d = nc.declare_dram_parameter("tout", [P, D], f32, isOutput=True)
    with TileContext(nc) as tc:
        with tc.tile_pool(name="s", bufs=1) as sp:
            t = sp.tile([P, D], f32, tag="t")
            nc.sync.dma_start(t[:], a_d[:])
            nc.sync.dma_start(o_d[:], t[:])
    nc.finalize()
    _fixup_multiwait(nc)
    return nc


def profile(inputs, iters=15):
    nc = _get_nc()
    in_maps = _make_in_maps(inputs)
    times, out = _timed_runner(nc, in_maps, iters)

    tnc = _tiny_nc()
    tiny_maps = [{"a": np.zeros((P, D), np.float32)} for _ in range(M)]
    tiny_times, _ = _timed_runner(tnc, tiny_maps, iters)

    t_full = float(np.median(times))
    t_tiny = float(np.median(tiny_times))
    ns = t_full - t_tiny
    return out, ns, {
        "full_median_ns": t_full,
        "tiny_median_ns": t_tiny,
        "full_all": times,
        "tiny_all": tiny_times,
    }
